# revision 45
# baseline (speedup 1.0000x reference)
"""A3C ChebConv (K=3) GNN model as a distributed Bass kernel on 8 TRN2 cores.

Math restructuring: the reference computes
    Tx0 = x; Tx1 = L@x; Tx2 = 2*L@Tx1 - x
    out = Tx0@W0 + Tx1@W1 + Tx2@W2 + b
Since L acts on the node dim and W on the feature dim, they commute:
    out = Y0 - Y2 + b + L@(Y1 + 2*L@Y2),   Y_k = x@W_k
So the only big compute is x@W (feature contraction, F=65536), which is
sharded over F across 8 cores; the [100, 360] partial products are
all-reduced, and the tiny Laplacian/tanh/FC epilogue runs on every core.

Per-core device graph:
  - one fused matmul  xT_shard[8192,100].T @ W_shard[8192,360] -> PSUM[100,360]
    (64 K-tiles of 128, streamed from one interleaved DRAM buffer)
  - AllReduce[100,360] over 8 cores
  - epilogue: U = L@Y2; Vin = Y1 + 2U; Z = bias + L@Vin + Y0 - Y2;
    emb = tanh(Z); FC heads via 60 accumulating [K=100,M=2]x[K=100,N=101]
    matmuls + one extras/bias matmul -> out[2,101]
"""

import numpy as np

import concourse.bass as bass
import concourse.bacc as bacc
import concourse.mybir as mybir
from concourse import tile
from concourse.tile_rust import add_dep_helper

N_CORES = 8
N = 100          # nodes
F = 65536        # input features
FS = F // N_CORES  # features per core
C = 60           # conv channels per head
CB = 2 * C       # both heads interleaved [actor | critic] per Cheb order
NW = 6 * C       # 360 = fused W columns (3 cheb orders x 2 heads)
BB = NW + N      # 460 = bigbuf row: [W row | xT row]
KT = 128         # contraction tile
NKT = FS // KT   # 64 K tiles per core
CHUNKS = 8       # DMA chunks (NKT/CHUNKS tiles each)
TPC = NKT // CHUNKS
ACT = 100        # action dim
FCN = ACT + 1    # fused FC output cols: [logits | value]
SM_COLS = 224    # smalls tensor cols

F32 = mybir.dt.float32
F32R = mybir.dt.float32r
BF16 = mybir.dt.bfloat16
# Big-matmul operand mode.  "f32r": full f32 storage, float32r matmul
# (1 cycle/row at free-dim>=256, near-f32 accuracy).  "bf16": half the
# HBM traffic but ~1e-2 output error.  The runtime's ~44us collective
# entry barrier hides most of the stream either way, so f32r costs only
# a few us and buys ~4x accuracy margin.
MM_MODE = "f32r"
MM_BF16 = MM_MODE == "bf16"
MMDT = BF16 if MM_BF16 else F32R
# FC path: batch FCB channels per matmul so the free dim (FCB*101=505)
# clears the >=256 threshold where float32r runs at 1 cycle/row -- full
# f32 storage precision AND fewer matmul issues than per-channel bf16.
FCDT = F32R
FCB = 5                    # channels per FC matmul
FCS = C // FCB             # 12 accumulation steps
# +1 pad column: fp32r matmuls fail the ISA check with an odd free dim.
FCW_FREE = FCB * FCN + 1   # 506 floats = 2024B, fits one PSUM bank
# Wake the collective firmware early with a tiny dummy AllReduce that runs
# under the streaming phase, so the real AllReduce doesn't pay the ~11us
# ncfw wake latency.  (Measured: back-to-back collectives queue badly on
# this stack -- leave off.)
PREWARM_CC = False
# AllReduce dtype for the [100,360] partials: the stage time is ncfw
# per-step dominated, so bf16 doesn't help; keep f32 (exact).
AR_BF16 = False
# K-tiles per DMA chunk, front-loaded small so the TensorEngine starts early.
# Each dma_start costs ~0.7-1.1us of sequencer issue time, so chunk issues
# alternate between the two HWDGE engines (sync=SP, scalar=Activation).
CHUNK_SIZES = [2, 4, 8, 10, 10, 10, 10, 10]
assert sum(CHUNK_SIZES) == NKT


def build_nc(debug: bool = False, reps: int = 1):
    nc = bacc.Bacc(
        "TRN2", target_bir_lowering=False, debug=debug, num_devices=N_CORES
    )
    bigbuf = nc.dram_tensor("bigbuf", [128, NKT * BB], MMDT, kind="ExternalInput")
    fcw = nc.dram_tensor("fcw", [N, FCS * FCW_FREE], FCDT, kind="ExternalInput")
    lt = nc.dram_tensor("lt", [N, N], F32, kind="ExternalInput")
    smalls = nc.dram_tensor("smalls", [4, SM_COLS], F32, kind="ExternalInput")
    out_ext = nc.dram_tensor("out", [2, FCN], F32, kind="ExternalOutput")

    with tile.TileContext(nc) as tc:
        with (
            tc.tile_pool(name="big", bufs=1) as bigpool,
            tc.tile_pool(name="wk", bufs=1) as wk,
            tc.tile_pool(name="ps", bufs=1, space="PSUM") as ps,
            tc.tile_pool(name="dram", bufs=1, space="DRAM") as dram,
        ):
            for _rep in range(reps):
                _build_body(nc, bigpool, wk, ps, dram, bigbuf, fcw, lt, smalls, out_ext)

    nc.compile()
    return nc


def _build_body(nc, bigpool, wk, ps, dram, bigbuf, fcw, lt, smalls, out_ext):
    if True:
        if True:
            # Small persistent tensors, issued on the scalar HWDGE queue and
            # order-pinned into the middle of the DMA ramp: early enough to
            # land before the AllReduce window (they'd contend with the
            # collective's SDMA traffic and stall the FC phase), late enough
            # not to delay the first streaming chunk.
            fcw_s = wk.tile([N, FCS * FCW_FREE], FCDT, tag="fcw")
            i_fcw = nc.scalar.dma_start(fcw_s[:], fcw[:, :])
            lt_s = wk.tile([N, N], F32, tag="lt")
            i_lt = nc.scalar.dma_start(lt_s[:], lt[:, :])
            sm_s = wk.tile([4, SM_COLS], F32, tag="smalls")
            i_sm = nc.scalar.dma_start(sm_s[:], smalls[:, :])
            ones_s = wk.tile([1, N], F32, tag="ones")
            nc.any.memset(ones_s[:], 1.0)

            if PREWARM_CC:
                warm_in = dram.tile([1, 8], F32, tag="warmin")
                warm_out = dram.tile([1, 8], F32, tag="warmout")
                warm_sb = wk.tile([1, 8], F32, tag="warmsb")
                nc.any.memset(warm_sb[:], 0.0)
                nc.gpsimd.dma_start(warm_in[:], warm_sb[:])
                nc.gpsimd.collective_compute(
                    "AllReduce",
                    mybir.AluOpType.add,
                    replica_groups=[list(range(N_CORES))],
                    ins=[warm_in.opt()],
                    outs=[warm_out.opt()],
                )

            # Big fused matmul: accumulate all 64 K-tiles into one PSUM bank
            psum_y = ps.tile([N, NW], F32, tag="y")
            mm = 0
            lo = 0
            chunk_dmas = []
            for ch, tpc in enumerate(CHUNK_SIZES):
                bt = bigpool.tile([128, tpc * BB], MMDT, tag=f"chunk{ch}")
                eng = nc.sync if ch % 2 == 0 else nc.scalar
                i_ch = eng.dma_start(bt[:], bigbuf[:, lo : lo + tpc * BB])
                chunk_dmas.append(i_ch)
                lo += tpc * BB
                for t in range(tpc):
                    base = t * BB
                    lhsT = bt[:, base + NW : base + BB]  # [128, 100] xT tile
                    rhs = bt[:, base : base + NW]        # [128, 360] W tile
                    nc.tensor.matmul(
                        psum_y[:], lhsT, rhs,
                        start=(mm == 0), stop=(mm == NKT - 1),
                    )
                    mm += 1
            # Order the scalar-queue issues: chunk1, chunk3, then the
            # persistent tensors, then the remaining odd chunks.
            scalar_order = [
                chunk_dmas[1], chunk_dmas[3], i_fcw, i_lt, i_sm,
                chunk_dmas[5], chunk_dmas[7],
            ]
            for a, b in zip(scalar_order[1:], scalar_order):
                add_dep_helper(a.ins, b.ins, False, "scalar DMA queue order")

            # Evict partials and AllReduce across the 8 cores
            ardt = BF16 if AR_BF16 else F32
            y_part = wk.tile([N, NW], ardt, tag="ypart")
            nc.vector.tensor_copy(y_part[:], psum_y[:])
            ar_in = dram.tile([N, NW], ardt, tag="arin")
            ar_out = dram.tile([N, NW], ardt, tag="arout")
            nc.sync.dma_start(ar_in[:], y_part[:])
            nc.gpsimd.collective_compute(
                "AllReduce",
                mybir.AluOpType.add,
                replica_groups=[list(range(N_CORES))],
                ins=[ar_in.opt()],
                outs=[ar_out.opt()],
            )
            if AR_BF16:
                y_in = wk.tile([N, NW], ardt, tag="yin")
                nc.sync.dma_start(y_in[:], ar_out[:])
                y_s = wk.tile([N, NW], F32, tag="ysb")
                nc.vector.tensor_copy(y_s[:], y_in[:])
            else:
                y_s = wk.tile([N, NW], F32, tag="ysb")
                nc.sync.dma_start(y_s[:], ar_out[:])

            # Epilogue: U = L@Y2
            psum_u = ps.tile([N, CB], F32, tag="u")
            nc.tensor.matmul(
                psum_u[:], lt_s[:], y_s[:, 2 * CB : 3 * CB], start=True, stop=True
            )
            # Vin = 2*U + Y1
            vin_s = wk.tile([N, CB], F32, tag="vin")
            nc.vector.scalar_tensor_tensor(
                vin_s[:], psum_u[:], 2.0, y_s[:, CB : 2 * CB],
                op0=mybir.AluOpType.mult, op1=mybir.AluOpType.add,
            )
            # Z = bias (ones[1,100].T @ biasrow[1,120]) + L@Vin
            psum_z = ps.tile([N, CB], F32, tag="z")
            nc.tensor.matmul(
                psum_z[:], ones_s[:], sm_s[0:1, 103 : 103 + CB],
                start=True, stop=False, skip_group_check=True,
            )
            nc.tensor.matmul(
                psum_z[:], lt_s[:], vin_s[:],
                start=False, stop=True, skip_group_check=True,
            )
            # emb = tanh(Z + Y0 - Y2)
            d_s = wk.tile([N, CB], F32, tag="d")
            nc.vector.tensor_sub(d_s[:], y_s[:, 0:CB], y_s[:, 2 * CB : 3 * CB])
            z_s = wk.tile([N, CB], F32, tag="zs")
            nc.vector.tensor_add(z_s[:], d_s[:], psum_z[:])
            emb_s = wk.tile([N, CB], F32R, tag="emb")
            nc.scalar.activation(
                emb_s[:], z_s[:], mybir.ActivationFunctionType.Tanh
            )

            # FC heads, FCB channels per accumulating matmul:
            # lhsT = emb[:, (c, 60+c) for c in step] [100, FCB*2]
            # rhs  = fcw[:, step block]              [100, FCB*101]
            # psum block j ([2j:2j+2, j*101:(j+1)*101]) accumulates the
            # (actor, critic) FC partials of channels c = j mod FCB.
            psum_fc = ps.tile([2 * FCB, FCW_FREE], F32, tag="fc")
            for s in range(FCS):
                lhsT = emb_s[:, 2 * FCB * s : 2 * FCB * (s + 1)]
                rhs = fcw_s[:, s * FCW_FREE : (s + 1) * FCW_FREE]
                nc.tensor.matmul(
                    psum_fc[:], lhsT, rhs,
                    start=(s == 0), stop=(s == FCS - 1), skip_group_check=True,
                )
            # extras + bias: lhsT = smalls[:,0:2] [K=4,M=2], rhs = smalls[:,2:103]
            psum_fce = ps.tile([2, FCN], F32, tag="fce")
            nc.tensor.matmul(
                psum_fce[:], sm_s[:, 0:2], sm_s[:, 2 : 2 + FCN],
                start=True, stop=True, skip_group_check=True,
            )
            # Sum the FCB diagonal blocks + extras.  Engine accesses must
            # start at partition 0, so evict PSUM to SBUF, then DMA-fold
            # rows {h, h+2, ...} onto partition h (strided partition reads
            # are fine for DMA), leaving block j at free offset j*505.
            g_s = wk.tile([2 * FCB, FCW_FREE], F32, tag="gs")
            nc.vector.tensor_copy(g_s[:], psum_fc[:])
            g2 = wk.tile([2, FCB * FCW_FREE], F32, tag="g2")
            nc.sync.dma_start(g2[0:1, :], g_s[0 : 2 * FCB : 2, :])
            nc.scalar.dma_start(g2[1:2, :], g_s[1 : 2 * FCB : 2, :])
            fc_s = wk.tile([2, FCN], F32, tag="fcs")
            nc.vector.tensor_copy(fc_s[:], psum_fce[:])
            for j in range(FCB):
                lo = j * FCW_FREE + j * FCN
                nc.vector.tensor_add(
                    fc_s[:], fc_s[:], g2[:, lo : lo + FCN]
                )
            nc.sync.dma_start(out_ext[:, :], fc_s[:])


def prepare_inputs(
    substrate_features, edge_index, v_cpu_demand_t, v_bw_demand_t,
    num_pending_v_nodes_t, actor_w, actor_b, critic_w, critic_b,
    actor_fc_w, actor_fc_b, critic_fc_w, critic_fc_b,
):
    """Host-side sharding / layout prep. Returns in_maps for the 8 cores."""
    x2 = np.asarray(substrate_features, np.float32)[0]        # [100, F]
    ei = np.asarray(edge_index).astype(np.int64)              # [2, E]
    aw = np.asarray(actor_w, np.float32)                      # [3, F, 60]
    ab = np.asarray(actor_b, np.float32)
    cw = np.asarray(critic_w, np.float32)
    cb = np.asarray(critic_b, np.float32)
    afw = np.asarray(actor_fc_w, np.float32)                  # [6003, 100]
    afb = np.asarray(actor_fc_b, np.float32)
    cfw = np.asarray(critic_fc_w, np.float32)                 # [6003, 1]
    cfb = np.asarray(critic_fc_b, np.float32)
    extras = [
        float(np.asarray(v_cpu_demand_t).reshape(-1)[0]),
        float(np.asarray(v_bw_demand_t).reshape(-1)[0]),
        float(np.asarray(num_pending_v_nodes_t).reshape(-1)[0]),
    ]

    # Dense scaled Laplacian from the edge list (PyG ChebConv, lambda_max=2)
    src, dst = ei[0], ei[1]
    deg = np.bincount(src, minlength=N).astype(np.float32)
    dis = np.where(deg > 0, 1.0 / np.sqrt(np.where(deg > 0, deg, 1.0)), 0.0)
    norm = -(dis[src] * dis[dst]).astype(np.float32)
    L = np.zeros((N, N), np.float32)
    np.add.at(L, (dst, src), norm)
    ltT = np.ascontiguousarray(L.T)                            # lhsT layout

    # Fused conv weights [F, 360]: three Cheb-order blocks of 120 columns;
    # within a block, actor/critic channels pairwise interleaved
    # [a0, c0, a1, c1, ...] so FC lhsT slices of emb are contiguous.
    w_all = np.empty((F, 3, C, 2), np.float32)
    for k in range(3):
        w_all[:, k, :, 0] = aw[k]
        w_all[:, k, :, 1] = cw[k]
    w_all = w_all.reshape(F, NW)
    xT = np.ascontiguousarray(x2.T)                            # [F, 100]

    # FC weights rearranged: fcw[n, c*101 + a] = actor_fc_w[n*60+c, a],
    # col 100 = critic_fc_w[n*60+c, 0]
    A = afw[:6000].reshape(N, C, ACT)
    Cc = cfw[:6000].reshape(N, C, 1)
    fcw_raw = np.concatenate([A, Cc], axis=2).reshape(N, FCS, FCB * FCN)
    fcw_host = np.zeros((N, FCS, FCW_FREE), np.float32)
    fcw_host[:, :, : FCB * FCN] = fcw_raw
    fcw_host = np.ascontiguousarray(fcw_host.reshape(N, FCS * FCW_FREE))
    if FCDT == BF16:
        import ml_dtypes

        fcw_host = fcw_host.astype(ml_dtypes.bfloat16)

    # smalls [4, 224]:
    #  [:, 0:2]      extras lhsT columns (both identical): [v_cpu, v_bw, n_pend, 1]
    #  [:, 2:103]    extras rhs rows: actor_fc_w[6000+j]|critic_fc_w[6000+j];
    #                row 3 = [actor_fc_b | critic_fc_b]
    #  [0, 103:223]  conv bias row [actor_b | critic_b]
    smalls = np.zeros((4, SM_COLS), np.float32)
    for j in range(3):
        smalls[j, 0:2] = extras[j]
        smalls[j, 2 : 2 + ACT] = afw[6000 + j]
        smalls[j, 2 + ACT] = cfw[6000 + j, 0]
    smalls[3, 0:2] = 1.0
    smalls[3, 2 : 2 + ACT] = afb
    smalls[3, 2 + ACT] = cfb[0]
    smalls[0, 103 : 103 + CB] = np.stack([ab, cb], axis=1).reshape(-1)

    in_maps = []
    for m in range(N_CORES):
        sl = slice(m * FS, (m + 1) * FS)
        big = np.concatenate([w_all[sl], xT[sl]], axis=1)      # [8192, 460]
        big = np.ascontiguousarray(
            big.reshape(NKT, KT, BB).transpose(1, 0, 2).reshape(128, NKT * BB)
        )
        if MM_BF16:
            import ml_dtypes

            big = big.astype(ml_dtypes.bfloat16)
        in_maps.append(
            {"bigbuf": big, "fcw": fcw_host, "lt": ltT, "smalls": smalls}
        )
    return in_maps


def unshard(results):
    out = np.asarray(results[0]["out"], np.float32)            # [2, 101]
    logits = np.ascontiguousarray(out[0:1, 0:ACT])             # [1, 100]
    values = np.ascontiguousarray(out[1:2, ACT : ACT + 1])     # [1, 1]
    return logits, values


_CACHED = {}


def kernel(**inputs):
    from concourse.bass_utils import run_bass_kernel_spmd

    in_maps = prepare_inputs(**inputs)
    if "nc" not in _CACHED:
        _CACHED["nc"] = build_nc(debug=False)
    res = run_bass_kernel_spmd(
        _CACHED["nc"], in_maps, core_ids=list(range(N_CORES))
    )
    return unshard(res.results)


def run_profiled(in_maps, tmpdir=None, trace=False):
    """Like kernel(), but optionally with NTFF profiling."""
    from concourse.bass_utils import run_bass_kernel_spmd

    if "nc" not in _CACHED:
        _CACHED["nc"] = build_nc(debug=False)
    res = run_bass_kernel_spmd(
        _CACHED["nc"], in_maps, core_ids=list(range(N_CORES)),
        trace=trace, tmpdir=tmpdir,
    )
    return unshard(res.results), res.exec_time_ns, res


# revision 47
# speedup vs baseline: 1.1166x; 1.1166x over previous
"""A3C ChebConv (K=3) GNN model as a distributed Bass kernel on 8 TRN2 cores.

Math restructuring: the reference computes
    Tx0 = x; Tx1 = L@x; Tx2 = 2*L@Tx1 - x
    out = Tx0@W0 + Tx1@W1 + Tx2@W2 + b
Since L acts on the node dim and W on the feature dim, they commute:
    out = Y0 - Y2 + b + L@(Y1 + 2*L@Y2),   Y_k = x@W_k
So the only big compute is x@W (feature contraction, F=65536), which is
sharded over F across 8 cores; the [100, 360] partial products are
all-reduced, and the tiny Laplacian/tanh/FC epilogue runs on every core.

Per-core device graph:
  - one fused matmul  xT_shard[8192,100].T @ W_shard[8192,360] -> PSUM[100,360]
    (64 K-tiles of 128, streamed from one interleaved DRAM buffer)
  - AllReduce[100,360] over 8 cores
  - epilogue: U = L@Y2; Vin = Y1 + 2U; Z = bias + L@Vin + Y0 - Y2;
    emb = tanh(Z); FC heads via 60 accumulating [K=100,M=2]x[K=100,N=101]
    matmuls + one extras/bias matmul -> out[2,101]
"""

import numpy as np

import concourse.bass as bass
import concourse.bacc as bacc
import concourse.mybir as mybir
from concourse import tile
from concourse.tile_rust import add_dep_helper

N_CORES = 8
N = 100          # nodes
F = 65536        # input features
FS = F // N_CORES  # features per core
C = 60           # conv channels per head
CB = 2 * C       # both heads interleaved [actor | critic] per Cheb order
NW = 6 * C       # 360 = fused W columns (3 cheb orders x 2 heads)
BB = NW + N      # 460 = bigbuf row: [W row | xT row]
KT = 128         # contraction tile
NKT = FS // KT   # 64 K tiles per core
CHUNKS = 8       # DMA chunks (NKT/CHUNKS tiles each)
TPC = NKT // CHUNKS
ACT = 100        # action dim
FCN = ACT + 1    # fused FC output cols: [logits | value]
SM_COLS = 224    # smalls tensor cols

F32 = mybir.dt.float32
F32R = mybir.dt.float32r
BF16 = mybir.dt.bfloat16
# Big-matmul operand mode.  "f32r": full f32 storage, float32r matmul
# (1 cycle/row at free-dim>=256, near-f32 accuracy).  "bf16": half the
# HBM traffic but ~1e-2 output error.  The runtime's ~44us collective
# entry barrier hides most of the stream either way, so f32r costs only
# a few us and buys ~4x accuracy margin.
MM_MODE = "f32r"
MM_BF16 = MM_MODE == "bf16"
MMDT = BF16 if MM_BF16 else F32R
# FC path: batch FCB channels per matmul so the free dim (FCB*101=505)
# clears the >=256 threshold where float32r runs at 1 cycle/row -- full
# f32 storage precision AND fewer matmul issues than per-channel bf16.
FCDT = F32R
FCB = 5                    # channels per FC matmul
FCS = C // FCB             # 12 accumulation steps
# +1 pad column: fp32r matmuls fail the ISA check with an odd free dim.
FCW_FREE = FCB * FCN + 1   # 506 floats = 2024B, fits one PSUM bank
# Wake the collective firmware early with a tiny dummy AllReduce that runs
# under the streaming phase, so the real AllReduce doesn't pay the ~11us
# ncfw wake latency.  (Measured: back-to-back collectives queue badly on
# this stack -- leave off.)
PREWARM_CC = False
# AllReduce dtype for the [100,360] partials: the stage time is ncfw
# per-step dominated, so bf16 doesn't help; keep f32 (exact).
AR_BF16 = False
# K-tiles per DMA chunk: ramp up (TensorEngine starts early) and back down
# (short last-byte -> AllReduce-doorbell path).  Each dma_start costs
# ~0.7-1.1us of sequencer issue time, so chunk issues alternate between the
# two HWDGE engines (sync=SP, scalar=Activation).
CHUNK_SIZES = [2, 4, 8, 10, 12, 12, 8, 4, 4]
assert sum(CHUNK_SIZES) == NKT


def build_nc(debug: bool = False, reps: int = 1):
    nc = bacc.Bacc(
        "TRN2", target_bir_lowering=False, debug=debug, num_devices=N_CORES
    )
    bigbuf = nc.dram_tensor("bigbuf", [128, NKT * BB], MMDT, kind="ExternalInput")
    fcw = nc.dram_tensor("fcw", [N, FCS * FCW_FREE], FCDT, kind="ExternalInput")
    lt = nc.dram_tensor("lt", [N, N], F32, kind="ExternalInput")
    smalls = nc.dram_tensor("smalls", [4, SM_COLS], F32, kind="ExternalInput")
    out_ext = nc.dram_tensor("out", [2, FCN], F32, kind="ExternalOutput")

    with tile.TileContext(nc) as tc:
        with (
            tc.tile_pool(name="big", bufs=1) as bigpool,
            tc.tile_pool(name="wk", bufs=1) as wk,
            tc.tile_pool(name="ps", bufs=1, space="PSUM") as ps,
            tc.tile_pool(name="dram", bufs=1, space="DRAM") as dram,
        ):
            for _rep in range(reps):
                _build_body(nc, bigpool, wk, ps, dram, bigbuf, fcw, lt, smalls, out_ext)

    nc.compile()
    return nc


def _build_body(nc, bigpool, wk, ps, dram, bigbuf, fcw, lt, smalls, out_ext):
    if True:
        if True:
            # Small persistent tensors, issued on the scalar HWDGE queue and
            # order-pinned into the middle of the DMA ramp: early enough to
            # land before the AllReduce window (they'd contend with the
            # collective's SDMA traffic and stall the FC phase), late enough
            # not to delay the first streaming chunk.
            fcw_s = wk.tile([N, FCS * FCW_FREE], FCDT, tag="fcw")
            i_fcw = nc.scalar.dma_start(fcw_s[:], fcw[:, :])
            lt_s = wk.tile([N, N], F32, tag="lt")
            i_lt = nc.scalar.dma_start(lt_s[:], lt[:, :])
            sm_s = wk.tile([4, SM_COLS], F32, tag="smalls")
            i_sm = nc.scalar.dma_start(sm_s[:], smalls[:, :])
            ones_s = wk.tile([1, N], F32, tag="ones")
            nc.any.memset(ones_s[:], 1.0)

            if PREWARM_CC:
                warm_in = dram.tile([1, 8], F32, tag="warmin")
                warm_out = dram.tile([1, 8], F32, tag="warmout")
                warm_sb = wk.tile([1, 8], F32, tag="warmsb")
                nc.any.memset(warm_sb[:], 0.0)
                nc.gpsimd.dma_start(warm_in[:], warm_sb[:])
                nc.gpsimd.collective_compute(
                    "AllReduce",
                    mybir.AluOpType.add,
                    replica_groups=[list(range(N_CORES))],
                    ins=[warm_in.opt()],
                    outs=[warm_out.opt()],
                )

            # Big fused matmul: accumulate all 64 K-tiles into one PSUM bank
            psum_y = ps.tile([N, NW], F32, tag="y")
            mm = 0
            lo = 0
            chunk_dmas = []
            for ch, tpc in enumerate(CHUNK_SIZES):
                bt = bigpool.tile([128, tpc * BB], MMDT, tag=f"chunk{ch}")
                eng = nc.sync if ch % 2 == 0 else nc.scalar
                i_ch = eng.dma_start(bt[:], bigbuf[:, lo : lo + tpc * BB])
                chunk_dmas.append(i_ch)
                lo += tpc * BB
                for t in range(tpc):
                    base = t * BB
                    lhsT = bt[:, base + NW : base + BB]  # [128, 100] xT tile
                    rhs = bt[:, base : base + NW]        # [128, 360] W tile
                    nc.tensor.matmul(
                        psum_y[:], lhsT, rhs,
                        start=(mm == 0), stop=(mm == NKT - 1),
                    )
                    mm += 1
            # Order the scalar-queue issues: small lt/smalls early (the
            # Z-bias matmul wants them), streaming chunks next, and the
            # 2.4MB fcw LAST -- it isn't consumed until the FC phase after
            # the AllReduce, and every byte moved before the doorbell
            # delays the collective on all 8 cores.
            scalar_order = [
                chunk_dmas[1], i_lt, i_sm, chunk_dmas[3],
                chunk_dmas[5], chunk_dmas[7], i_fcw,
            ]
            for a, b in zip(scalar_order[1:], scalar_order):
                add_dep_helper(a.ins, b.ins, False, "scalar DMA queue order")

            # Evict partials and AllReduce across the 8 cores
            ardt = BF16 if AR_BF16 else F32
            y_part = wk.tile([N, NW], ardt, tag="ypart")
            nc.vector.tensor_copy(y_part[:], psum_y[:])
            ar_in = dram.tile([N, NW], ardt, tag="arin")
            ar_out = dram.tile([N, NW], ardt, tag="arout")
            nc.sync.dma_start(ar_in[:], y_part[:])
            nc.gpsimd.collective_compute(
                "AllReduce",
                mybir.AluOpType.add,
                replica_groups=[list(range(N_CORES))],
                ins=[ar_in.opt()],
                outs=[ar_out.opt()],
            )
            if AR_BF16:
                y_in = wk.tile([N, NW], ardt, tag="yin")
                nc.sync.dma_start(y_in[:], ar_out[:])
                y_s = wk.tile([N, NW], F32, tag="ysb")
                nc.vector.tensor_copy(y_s[:], y_in[:])
            else:
                y_s = wk.tile([N, NW], F32, tag="ysb")
                nc.sync.dma_start(y_s[:], ar_out[:])

            # Epilogue: U = L@Y2
            psum_u = ps.tile([N, CB], F32, tag="u")
            nc.tensor.matmul(
                psum_u[:], lt_s[:], y_s[:, 2 * CB : 3 * CB], start=True, stop=True
            )
            # Vin = 2*U + Y1
            vin_s = wk.tile([N, CB], F32, tag="vin")
            nc.vector.scalar_tensor_tensor(
                vin_s[:], psum_u[:], 2.0, y_s[:, CB : 2 * CB],
                op0=mybir.AluOpType.mult, op1=mybir.AluOpType.add,
            )
            # Z = bias (ones[1,100].T @ biasrow[1,120]) + L@Vin
            psum_z = ps.tile([N, CB], F32, tag="z")
            nc.tensor.matmul(
                psum_z[:], ones_s[:], sm_s[0:1, 103 : 103 + CB],
                start=True, stop=False, skip_group_check=True,
            )
            nc.tensor.matmul(
                psum_z[:], lt_s[:], vin_s[:],
                start=False, stop=True, skip_group_check=True,
            )
            # emb = tanh(Z + Y0 - Y2)
            d_s = wk.tile([N, CB], F32, tag="d")
            nc.vector.tensor_sub(d_s[:], y_s[:, 0:CB], y_s[:, 2 * CB : 3 * CB])
            z_s = wk.tile([N, CB], F32, tag="zs")
            nc.vector.tensor_add(z_s[:], d_s[:], psum_z[:])
            emb_s = wk.tile([N, CB], F32R, tag="emb")
            nc.scalar.activation(
                emb_s[:], z_s[:], mybir.ActivationFunctionType.Tanh
            )

            # FC heads, FCB channels per accumulating matmul:
            # lhsT = emb[:, (c, 60+c) for c in step] [100, FCB*2]
            # rhs  = fcw[:, step block]              [100, FCB*101]
            # psum block j ([2j:2j+2, j*101:(j+1)*101]) accumulates the
            # (actor, critic) FC partials of channels c = j mod FCB.
            psum_fc = ps.tile([2 * FCB, FCW_FREE], F32, tag="fc")
            for s in range(FCS):
                lhsT = emb_s[:, 2 * FCB * s : 2 * FCB * (s + 1)]
                rhs = fcw_s[:, s * FCW_FREE : (s + 1) * FCW_FREE]
                nc.tensor.matmul(
                    psum_fc[:], lhsT, rhs,
                    start=(s == 0), stop=(s == FCS - 1), skip_group_check=True,
                )
            # extras + bias: lhsT = smalls[:,0:2] [K=4,M=2], rhs = smalls[:,2:103]
            psum_fce = ps.tile([2, FCN], F32, tag="fce")
            nc.tensor.matmul(
                psum_fce[:], sm_s[:, 0:2], sm_s[:, 2 : 2 + FCN],
                start=True, stop=True, skip_group_check=True,
            )
            # Sum the FCB diagonal blocks + extras.  Engine accesses must
            # start at partition 0, so evict PSUM to SBUF, then DMA-fold
            # rows {h, h+2, ...} onto partition h (strided partition reads
            # are fine for DMA), leaving block j at free offset j*505.
            g_s = wk.tile([2 * FCB, FCW_FREE], F32, tag="gs")
            nc.vector.tensor_copy(g_s[:], psum_fc[:])
            g2 = wk.tile([2, FCB * FCW_FREE], F32, tag="g2")
            nc.sync.dma_start(g2[0:1, :], g_s[0 : 2 * FCB : 2, :])
            nc.scalar.dma_start(g2[1:2, :], g_s[1 : 2 * FCB : 2, :])
            fc_s = wk.tile([2, FCN], F32, tag="fcs")
            nc.vector.tensor_copy(fc_s[:], psum_fce[:])
            for j in range(FCB):
                lo = j * FCW_FREE + j * FCN
                nc.vector.tensor_add(
                    fc_s[:], fc_s[:], g2[:, lo : lo + FCN]
                )
            nc.sync.dma_start(out_ext[:, :], fc_s[:])


def prepare_inputs(
    substrate_features, edge_index, v_cpu_demand_t, v_bw_demand_t,
    num_pending_v_nodes_t, actor_w, actor_b, critic_w, critic_b,
    actor_fc_w, actor_fc_b, critic_fc_w, critic_fc_b,
):
    """Host-side sharding / layout prep. Returns in_maps for the 8 cores."""
    x2 = np.asarray(substrate_features, np.float32)[0]        # [100, F]
    ei = np.asarray(edge_index).astype(np.int64)              # [2, E]
    aw = np.asarray(actor_w, np.float32)                      # [3, F, 60]
    ab = np.asarray(actor_b, np.float32)
    cw = np.asarray(critic_w, np.float32)
    cb = np.asarray(critic_b, np.float32)
    afw = np.asarray(actor_fc_w, np.float32)                  # [6003, 100]
    afb = np.asarray(actor_fc_b, np.float32)
    cfw = np.asarray(critic_fc_w, np.float32)                 # [6003, 1]
    cfb = np.asarray(critic_fc_b, np.float32)
    extras = [
        float(np.asarray(v_cpu_demand_t).reshape(-1)[0]),
        float(np.asarray(v_bw_demand_t).reshape(-1)[0]),
        float(np.asarray(num_pending_v_nodes_t).reshape(-1)[0]),
    ]

    # Dense scaled Laplacian from the edge list (PyG ChebConv, lambda_max=2)
    src, dst = ei[0], ei[1]
    deg = np.bincount(src, minlength=N).astype(np.float32)
    dis = np.where(deg > 0, 1.0 / np.sqrt(np.where(deg > 0, deg, 1.0)), 0.0)
    norm = -(dis[src] * dis[dst]).astype(np.float32)
    L = np.zeros((N, N), np.float32)
    np.add.at(L, (dst, src), norm)
    ltT = np.ascontiguousarray(L.T)                            # lhsT layout

    # Fused conv weights [F, 360]: three Cheb-order blocks of 120 columns;
    # within a block, actor/critic channels pairwise interleaved
    # [a0, c0, a1, c1, ...] so FC lhsT slices of emb are contiguous.
    w_all = np.empty((F, 3, C, 2), np.float32)
    for k in range(3):
        w_all[:, k, :, 0] = aw[k]
        w_all[:, k, :, 1] = cw[k]
    w_all = w_all.reshape(F, NW)
    xT = np.ascontiguousarray(x2.T)                            # [F, 100]

    # FC weights rearranged: fcw[n, c*101 + a] = actor_fc_w[n*60+c, a],
    # col 100 = critic_fc_w[n*60+c, 0]
    A = afw[:6000].reshape(N, C, ACT)
    Cc = cfw[:6000].reshape(N, C, 1)
    fcw_raw = np.concatenate([A, Cc], axis=2).reshape(N, FCS, FCB * FCN)
    fcw_host = np.zeros((N, FCS, FCW_FREE), np.float32)
    fcw_host[:, :, : FCB * FCN] = fcw_raw
    fcw_host = np.ascontiguousarray(fcw_host.reshape(N, FCS * FCW_FREE))
    if FCDT == BF16:
        import ml_dtypes

        fcw_host = fcw_host.astype(ml_dtypes.bfloat16)

    # smalls [4, 224]:
    #  [:, 0:2]      extras lhsT columns (both identical): [v_cpu, v_bw, n_pend, 1]
    #  [:, 2:103]    extras rhs rows: actor_fc_w[6000+j]|critic_fc_w[6000+j];
    #                row 3 = [actor_fc_b | critic_fc_b]
    #  [0, 103:223]  conv bias row [actor_b | critic_b]
    smalls = np.zeros((4, SM_COLS), np.float32)
    for j in range(3):
        smalls[j, 0:2] = extras[j]
        smalls[j, 2 : 2 + ACT] = afw[6000 + j]
        smalls[j, 2 + ACT] = cfw[6000 + j, 0]
    smalls[3, 0:2] = 1.0
    smalls[3, 2 : 2 + ACT] = afb
    smalls[3, 2 + ACT] = cfb[0]
    smalls[0, 103 : 103 + CB] = np.stack([ab, cb], axis=1).reshape(-1)

    in_maps = []
    for m in range(N_CORES):
        sl = slice(m * FS, (m + 1) * FS)
        big = np.concatenate([w_all[sl], xT[sl]], axis=1)      # [8192, 460]
        big = np.ascontiguousarray(
            big.reshape(NKT, KT, BB).transpose(1, 0, 2).reshape(128, NKT * BB)
        )
        if MM_BF16:
            import ml_dtypes

            big = big.astype(ml_dtypes.bfloat16)
        in_maps.append(
            {"bigbuf": big, "fcw": fcw_host, "lt": ltT, "smalls": smalls}
        )
    return in_maps


def unshard(results):
    out = np.asarray(results[0]["out"], np.float32)            # [2, 101]
    logits = np.ascontiguousarray(out[0:1, 0:ACT])             # [1, 100]
    values = np.ascontiguousarray(out[1:2, ACT : ACT + 1])     # [1, 1]
    return logits, values


_CACHED = {}


def kernel(**inputs):
    from concourse.bass_utils import run_bass_kernel_spmd

    in_maps = prepare_inputs(**inputs)
    if "nc" not in _CACHED:
        _CACHED["nc"] = build_nc(debug=False)
    res = run_bass_kernel_spmd(
        _CACHED["nc"], in_maps, core_ids=list(range(N_CORES))
    )
    return unshard(res.results)


def run_profiled(in_maps, tmpdir=None, trace=False):
    """Like kernel(), but optionally with NTFF profiling."""
    from concourse.bass_utils import run_bass_kernel_spmd

    if "nc" not in _CACHED:
        _CACHED["nc"] = build_nc(debug=False)
    res = run_bass_kernel_spmd(
        _CACHED["nc"], in_maps, core_ids=list(range(N_CORES)),
        trace=trace, tmpdir=tmpdir,
    )
    return unshard(res.results), res.exec_time_ns, res


# revision 51
# speedup vs baseline: 1.2804x; 1.1467x over previous
"""A3C ChebConv (K=3) GNN model as a distributed Bass kernel on 8 TRN2 cores.

Math restructuring: the reference computes
    Tx0 = x; Tx1 = L@x; Tx2 = 2*L@Tx1 - x
    out = Tx0@W0 + Tx1@W1 + Tx2@W2 + b
Since L acts on the node dim and W on the feature dim, they commute:
    out = Y0 - Y2 + b + L@(Y1 + 2*L@Y2),   Y_k = x@W_k
So the only big compute is x@W (feature contraction, F=65536), which is
sharded over F across 8 cores; the [100, 360] partial products are
all-reduced, and the tiny Laplacian/tanh/FC epilogue runs on every core.

Per-core device graph:
  - one fused matmul  xT_shard[8192,100].T @ W_shard[8192,360] -> PSUM[100,360]
    (64 K-tiles of 128, streamed from one interleaved DRAM buffer)
  - AllReduce[100,360] over 8 cores
  - epilogue: U = L@Y2; Vin = Y1 + 2U; Z = bias + L@Vin + Y0 - Y2;
    emb = tanh(Z); FC heads via 60 accumulating [K=100,M=2]x[K=100,N=101]
    matmuls + one extras/bias matmul -> out[2,101]
"""

import numpy as np

import concourse.bass as bass
import concourse.bacc as bacc
import concourse.mybir as mybir
from concourse import tile
from concourse.tile_rust import add_dep_helper

N_CORES = 8
N = 100          # nodes
F = 65536        # input features
FS = F // N_CORES  # features per core
C = 60           # conv channels per head
CB = 2 * C       # both heads interleaved [actor | critic] per Cheb order
NW = 6 * C       # 360 = fused W columns (3 cheb orders x 2 heads)
BB = NW + N      # 460 = bigbuf row: [W row | xT row]
KT = 128         # contraction tile
NKT = FS // KT   # 64 K tiles per core
CHUNKS = 8       # DMA chunks (NKT/CHUNKS tiles each)
TPC = NKT // CHUNKS
ACT = 100        # action dim
FCN = ACT + 1    # fused FC output cols: [logits | value]
SM_COLS = 224    # smalls tensor cols

F32 = mybir.dt.float32
F32R = mybir.dt.float32r
BF16 = mybir.dt.bfloat16
# Big-matmul operand mode.  "f32r": full f32 storage, float32r matmul
# (1 cycle/row at free-dim>=256, near-f32 accuracy).  "bf16": half the
# HBM traffic but ~1e-2 output error.  The runtime's ~44us collective
# entry barrier hides most of the stream either way, so f32r costs only
# a few us and buys ~4x accuracy margin.
MM_MODE = "f32r"
MM_BF16 = MM_MODE == "bf16"
MMDT = BF16 if MM_BF16 else F32R
# FC path: batch FCB channels per matmul so the free dim (FCB*101=505)
# clears the >=256 threshold where float32r runs at 1 cycle/row -- full
# f32 storage precision AND fewer matmul issues than per-channel bf16.
FCDT = F32R
FCB = 5                    # channels per FC matmul
FCS = C // FCB             # 12 accumulation steps
# +1 pad column: fp32r matmuls fail the ISA check with an odd free dim.
FCW_FREE = FCB * FCN + 1   # 506 floats = 2024B, fits one PSUM bank
# Wake the collective firmware early with a tiny dummy AllReduce that runs
# under the streaming phase, so the real AllReduce doesn't pay the ~11us
# ncfw wake latency.  (Measured: back-to-back collectives queue badly on
# this stack -- leave off.)
PREWARM_CC = False
# AllReduce dtype for the [100,360] partials: the stage time is ncfw
# per-step dominated, so bf16 doesn't help; keep f32 (exact).
AR_BF16 = False
# K-tiles per DMA chunk: ramp up (TensorEngine starts early) and back down
# (short last-byte -> AllReduce-doorbell path).  Each dma_start costs
# ~0.7-1.1us of sequencer issue time, so chunk issues alternate between the
# two HWDGE engines (sync=SP, scalar=Activation).
CHUNK_SIZES = [2, 4, 8, 10, 12, 12, 8, 4, 4]
assert sum(CHUNK_SIZES) == NKT


def build_nc(debug: bool = False, reps: int = 1):
    nc = bacc.Bacc(
        "TRN2", target_bir_lowering=False, debug=debug, num_devices=N_CORES
    )
    bigbuf = nc.dram_tensor("bigbuf", [128, NKT * BB], MMDT, kind="ExternalInput")
    fcw = nc.dram_tensor("fcw", [N, FCS * FCW_FREE], FCDT, kind="ExternalInput")
    lt = nc.dram_tensor("lt", [N, N], F32, kind="ExternalInput")
    smalls = nc.dram_tensor("smalls", [4, SM_COLS], F32, kind="ExternalInput")
    out_ext = nc.dram_tensor("out", [2, FCN], F32, kind="ExternalOutput")

    with tile.TileContext(nc) as tc:
        with (
            tc.tile_pool(name="big", bufs=1) as bigpool,
            tc.tile_pool(name="wk", bufs=1) as wk,
            tc.tile_pool(name="ps", bufs=1, space="PSUM") as ps,
            tc.tile_pool(name="dram", bufs=1, space="DRAM") as dram,
        ):
            for _rep in range(reps):
                _build_body(nc, bigpool, wk, ps, dram, bigbuf, fcw, lt, smalls, out_ext)

    nc.compile()
    return nc


def _build_body(nc, bigpool, wk, ps, dram, bigbuf, fcw, lt, smalls, out_ext):
    if True:
        if True:
            # Small persistent tensors, issued on the scalar HWDGE queue and
            # order-pinned into the middle of the DMA ramp: early enough to
            # land before the AllReduce window (they'd contend with the
            # collective's SDMA traffic and stall the FC phase), late enough
            # not to delay the first streaming chunk.
            fcw_s = wk.tile([N, FCS * FCW_FREE], FCDT, tag="fcw")
            i_fcw = nc.scalar.dma_start(fcw_s[:], fcw[:, :])
            lt_s = wk.tile([N, N], F32, tag="lt")
            i_lt = nc.scalar.dma_start(lt_s[:], lt[:, :])
            sm_s = wk.tile([4, SM_COLS], F32, tag="smalls")
            i_sm = nc.scalar.dma_start(sm_s[:], smalls[:, :])
            ones_s = wk.tile([1, N], F32, tag="ones")
            nc.any.memset(ones_s[:], 1.0)

            if PREWARM_CC:
                warm_in = dram.tile([1, 8], F32, tag="warmin")
                warm_out = dram.tile([1, 8], F32, tag="warmout")
                warm_sb = wk.tile([1, 8], F32, tag="warmsb")
                nc.any.memset(warm_sb[:], 0.0)
                nc.gpsimd.dma_start(warm_in[:], warm_sb[:])
                nc.gpsimd.collective_compute(
                    "AllReduce",
                    mybir.AluOpType.add,
                    replica_groups=[list(range(N_CORES))],
                    ins=[warm_in.opt()],
                    outs=[warm_out.opt()],
                )

            # Big fused matmul: accumulate all 64 K-tiles into one PSUM bank
            psum_y = ps.tile([N, NW], F32, tag="y")
            mm = 0
            lo = 0
            chunk_dmas = []
            for ch, tpc in enumerate(CHUNK_SIZES):
                bt = bigpool.tile([128, tpc * BB], MMDT, tag=f"chunk{ch}")
                eng = nc.sync if ch % 2 == 0 else nc.scalar
                i_ch = eng.dma_start(bt[:], bigbuf[:, lo : lo + tpc * BB])
                chunk_dmas.append(i_ch)
                lo += tpc * BB
                for t in range(tpc):
                    base = t * BB
                    lhsT = bt[:, base + NW : base + BB]  # [128, 100] xT tile
                    rhs = bt[:, base : base + NW]        # [128, 360] W tile
                    nc.tensor.matmul(
                        psum_y[:], lhsT, rhs,
                        start=(mm == 0), stop=(mm == NKT - 1),
                    )
                    mm += 1
            # Order the scalar-queue issues: small lt/smalls early (the
            # Z-bias matmul wants them), streaming chunks next, and the
            # 2.4MB fcw LAST -- it isn't consumed until the FC phase after
            # the AllReduce, and every byte moved before the doorbell
            # delays the collective on all 8 cores.
            scalar_order = [
                chunk_dmas[1], i_lt, i_sm, chunk_dmas[3],
                chunk_dmas[5], chunk_dmas[7], i_fcw,
            ]
            for a, b in zip(scalar_order[1:], scalar_order):
                add_dep_helper(a.ins, b.ins, False, "scalar DMA queue order")

            # The Chebyshev epilogue is linear, so it commutes with the
            # cross-core sum: compute Z_partial on each core's Y partials
            # BEFORE the collective, then AllReduce only [100,120] (48KB).
            # The bias row is non-zero only in core 0's input data.
            y_part = wk.tile([N, NW], F32, tag="ypart")
            nc.vector.tensor_copy(y_part[:], psum_y[:])
            # U = L@Y2p
            psum_u = ps.tile([N, CB], F32, tag="u")
            nc.tensor.matmul(
                psum_u[:], lt_s[:], y_part[:, 2 * CB : 3 * CB],
                start=True, stop=True,
            )
            # Vin = 2*U + Y1p
            vin_s = wk.tile([N, CB], F32, tag="vin")
            nc.vector.scalar_tensor_tensor(
                vin_s[:], psum_u[:], 2.0, y_part[:, CB : 2 * CB],
                op0=mybir.AluOpType.mult, op1=mybir.AluOpType.add,
            )
            # Z_partial = bias (core 0 only) + L@Vin (+ Y0p - Y2p)
            psum_z = ps.tile([N, CB], F32, tag="z")
            nc.tensor.matmul(
                psum_z[:], ones_s[:], sm_s[0:1, 103 : 103 + CB],
                start=True, stop=False, skip_group_check=True,
            )
            nc.tensor.matmul(
                psum_z[:], lt_s[:], vin_s[:],
                start=False, stop=True, skip_group_check=True,
            )
            d_s = wk.tile([N, CB], F32, tag="d")
            nc.vector.tensor_sub(
                d_s[:], y_part[:, 0:CB], y_part[:, 2 * CB : 3 * CB]
            )
            z_s = wk.tile([N, CB], F32, tag="zs")
            nc.vector.tensor_add(z_s[:], d_s[:], psum_z[:])

            # AllReduce the Z partials across the 8 cores
            ar_in = dram.tile([N, CB], F32, tag="arin")
            ar_out = dram.tile([N, CB], F32, tag="arout")
            nc.sync.dma_start(ar_in[:], z_s[:])
            nc.gpsimd.collective_compute(
                "AllReduce",
                mybir.AluOpType.add,
                replica_groups=[list(range(N_CORES))],
                ins=[ar_in.opt()],
                outs=[ar_out.opt()],
            )
            zall_s = wk.tile([N, CB], F32, tag="zall")
            nc.sync.dma_start(zall_s[:], ar_out[:])

            # emb = tanh(Z)
            emb_s = wk.tile([N, CB], F32R, tag="emb")
            nc.scalar.activation(
                emb_s[:], zall_s[:], mybir.ActivationFunctionType.Tanh
            )

            # FC heads, FCB channels per accumulating matmul:
            # lhsT = emb[:, (c, 60+c) for c in step] [100, FCB*2]
            # rhs  = fcw[:, step block]              [100, FCB*101]
            # psum block j ([2j:2j+2, j*101:(j+1)*101]) accumulates the
            # (actor, critic) FC partials of channels c = j mod FCB.
            psum_fc = ps.tile([2 * FCB, FCW_FREE], F32, tag="fc")
            for s in range(FCS):
                lhsT = emb_s[:, 2 * FCB * s : 2 * FCB * (s + 1)]
                rhs = fcw_s[:, s * FCW_FREE : (s + 1) * FCW_FREE]
                nc.tensor.matmul(
                    psum_fc[:], lhsT, rhs,
                    start=(s == 0), stop=(s == FCS - 1), skip_group_check=True,
                )
            # extras + bias: lhsT = smalls[:,0:2] [K=4,M=2], rhs = smalls[:,2:103]
            psum_fce = ps.tile([2, FCN], F32, tag="fce")
            nc.tensor.matmul(
                psum_fce[:], sm_s[:, 0:2], sm_s[:, 2 : 2 + FCN],
                start=True, stop=True, skip_group_check=True,
            )
            # Sum the FCB diagonal blocks + extras.  Engine accesses must
            # start at partition 0, so evict PSUM to SBUF, then DMA-fold
            # rows {h, h+2, ...} onto partition h (strided partition reads
            # are fine for DMA), leaving block j at free offset j*505.
            g_s = wk.tile([2 * FCB, FCW_FREE], F32, tag="gs")
            nc.vector.tensor_copy(g_s[:], psum_fc[:])
            g2 = wk.tile([2, FCB * FCW_FREE], F32, tag="g2")
            nc.sync.dma_start(g2[0:1, :], g_s[0 : 2 * FCB : 2, :])
            nc.scalar.dma_start(g2[1:2, :], g_s[1 : 2 * FCB : 2, :])
            fc_s = wk.tile([2, FCN], F32, tag="fcs")
            nc.vector.tensor_add(fc_s[:], g2[:, 0:FCN], psum_fce[:])
            for j in range(1, FCB):
                lo = j * FCW_FREE + j * FCN
                nc.vector.tensor_add(
                    fc_s[:], fc_s[:], g2[:, lo : lo + FCN]
                )
            nc.sync.dma_start(out_ext[:, :], fc_s[:])


def prepare_inputs(
    substrate_features, edge_index, v_cpu_demand_t, v_bw_demand_t,
    num_pending_v_nodes_t, actor_w, actor_b, critic_w, critic_b,
    actor_fc_w, actor_fc_b, critic_fc_w, critic_fc_b,
):
    """Host-side sharding / layout prep. Returns in_maps for the 8 cores."""
    x2 = np.asarray(substrate_features, np.float32)[0]        # [100, F]
    ei = np.asarray(edge_index).astype(np.int64)              # [2, E]
    aw = np.asarray(actor_w, np.float32)                      # [3, F, 60]
    ab = np.asarray(actor_b, np.float32)
    cw = np.asarray(critic_w, np.float32)
    cb = np.asarray(critic_b, np.float32)
    afw = np.asarray(actor_fc_w, np.float32)                  # [6003, 100]
    afb = np.asarray(actor_fc_b, np.float32)
    cfw = np.asarray(critic_fc_w, np.float32)                 # [6003, 1]
    cfb = np.asarray(critic_fc_b, np.float32)
    extras = [
        float(np.asarray(v_cpu_demand_t).reshape(-1)[0]),
        float(np.asarray(v_bw_demand_t).reshape(-1)[0]),
        float(np.asarray(num_pending_v_nodes_t).reshape(-1)[0]),
    ]

    # Dense scaled Laplacian from the edge list (PyG ChebConv, lambda_max=2)
    src, dst = ei[0], ei[1]
    deg = np.bincount(src, minlength=N).astype(np.float32)
    dis = np.where(deg > 0, 1.0 / np.sqrt(np.where(deg > 0, deg, 1.0)), 0.0)
    norm = -(dis[src] * dis[dst]).astype(np.float32)
    L = np.zeros((N, N), np.float32)
    np.add.at(L, (dst, src), norm)
    ltT = np.ascontiguousarray(L.T)                            # lhsT layout

    # Fused conv weights [F, 360]: three Cheb-order blocks of 120 columns;
    # within a block, actor/critic channels pairwise interleaved
    # [a0, c0, a1, c1, ...] so FC lhsT slices of emb are contiguous.
    w_all = np.empty((F, 3, C, 2), np.float32)
    for k in range(3):
        w_all[:, k, :, 0] = aw[k]
        w_all[:, k, :, 1] = cw[k]
    w_all = w_all.reshape(F, NW)
    xT = np.ascontiguousarray(x2.T)                            # [F, 100]

    # FC weights rearranged: fcw[n, c*101 + a] = actor_fc_w[n*60+c, a],
    # col 100 = critic_fc_w[n*60+c, 0]
    A = afw[:6000].reshape(N, C, ACT)
    Cc = cfw[:6000].reshape(N, C, 1)
    fcw_raw = np.concatenate([A, Cc], axis=2).reshape(N, FCS, FCB * FCN)
    fcw_host = np.zeros((N, FCS, FCW_FREE), np.float32)
    fcw_host[:, :, : FCB * FCN] = fcw_raw
    fcw_host = np.ascontiguousarray(fcw_host.reshape(N, FCS * FCW_FREE))
    if FCDT == BF16:
        import ml_dtypes

        fcw_host = fcw_host.astype(ml_dtypes.bfloat16)

    # smalls [4, 224]:
    #  [:, 0:2]      extras lhsT columns (both identical): [v_cpu, v_bw, n_pend, 1]
    #  [:, 2:103]    extras rhs rows: actor_fc_w[6000+j]|critic_fc_w[6000+j];
    #                row 3 = [actor_fc_b | critic_fc_b]
    #  [0, 103:223]  conv bias row [actor_b | critic_b]
    smalls = np.zeros((4, SM_COLS), np.float32)
    for j in range(3):
        smalls[j, 0:2] = extras[j]
        smalls[j, 2 : 2 + ACT] = afw[6000 + j]
        smalls[j, 2 + ACT] = cfw[6000 + j, 0]
    smalls[3, 0:2] = 1.0
    smalls[3, 2 : 2 + ACT] = afb
    smalls[3, 2 + ACT] = cfb[0]
    smalls[0, 103 : 103 + CB] = np.stack([ab, cb], axis=1).reshape(-1)

    in_maps = []
    for m in range(N_CORES):
        sl = slice(m * FS, (m + 1) * FS)
        big = np.concatenate([w_all[sl], xT[sl]], axis=1)      # [8192, 460]
        big = np.ascontiguousarray(
            big.reshape(NKT, KT, BB).transpose(1, 0, 2).reshape(128, NKT * BB)
        )
        if MM_BF16:
            import ml_dtypes

            big = big.astype(ml_dtypes.bfloat16)
        # The conv bias flows through the Z AllReduce -- only core 0 adds it.
        sm_m = smalls.copy()
        if m > 0:
            sm_m[0, 103 : 103 + CB] = 0.0
        in_maps.append(
            {"bigbuf": big, "fcw": fcw_host, "lt": ltT, "smalls": sm_m}
        )
    return in_maps


def unshard(results):
    out = np.asarray(results[0]["out"], np.float32)            # [2, 101]
    logits = np.ascontiguousarray(out[0:1, 0:ACT])             # [1, 100]
    values = np.ascontiguousarray(out[1:2, ACT : ACT + 1])     # [1, 1]
    return logits, values


_CACHED = {}


def kernel(**inputs):
    from concourse.bass_utils import run_bass_kernel_spmd

    in_maps = prepare_inputs(**inputs)
    if "nc" not in _CACHED:
        _CACHED["nc"] = build_nc(debug=False)
    res = run_bass_kernel_spmd(
        _CACHED["nc"], in_maps, core_ids=list(range(N_CORES))
    )
    return unshard(res.results)


def run_profiled(in_maps, tmpdir=None, trace=False):
    """Like kernel(), but optionally with NTFF profiling."""
    from concourse.bass_utils import run_bass_kernel_spmd

    if "nc" not in _CACHED:
        _CACHED["nc"] = build_nc(debug=False)
    res = run_bass_kernel_spmd(
        _CACHED["nc"], in_maps, core_ids=list(range(N_CORES)),
        trace=trace, tmpdir=tmpdir,
    )
    return unshard(res.results), res.exec_time_ns, res


# revision 53
# speedup vs baseline: 1.3097x; 1.0229x over previous
"""A3C ChebConv (K=3) GNN model as a distributed Bass kernel on 8 TRN2 cores.

Math restructuring: the reference computes
    Tx0 = x; Tx1 = L@x; Tx2 = 2*L@Tx1 - x
    out = Tx0@W0 + Tx1@W1 + Tx2@W2 + b
Since L acts on the node dim and W on the feature dim, they commute:
    out = Y0 - Y2 + b + L@(Y1 + 2*L@Y2),   Y_k = x@W_k
So the only big compute is x@W (feature contraction, F=65536), which is
sharded over F across 8 cores; the [100, 360] partial products are
all-reduced, and the tiny Laplacian/tanh/FC epilogue runs on every core.

Per-core device graph:
  - one fused matmul  xT_shard[8192,100].T @ W_shard[8192,360] -> PSUM[100,360]
    (64 K-tiles of 128, streamed from one interleaved DRAM buffer)
  - AllReduce[100,360] over 8 cores
  - epilogue: U = L@Y2; Vin = Y1 + 2U; Z = bias + L@Vin + Y0 - Y2;
    emb = tanh(Z); FC heads via 60 accumulating [K=100,M=2]x[K=100,N=101]
    matmuls + one extras/bias matmul -> out[2,101]
"""

import numpy as np

import concourse.bass as bass
import concourse.bacc as bacc
import concourse.mybir as mybir
from concourse import tile
from concourse.tile_rust import add_dep_helper

N_CORES = 8
N = 100          # nodes
F = 65536        # input features
FS = F // N_CORES  # features per core
C = 60           # conv channels per head
CB = 2 * C       # both heads interleaved [actor | critic] per Cheb order
NW = 6 * C       # 360 = fused W columns (3 cheb orders x 2 heads)
BB = NW + N      # 460 = bigbuf row: [W row | xT row]
KT = 128         # contraction tile
NKT = FS // KT   # 64 K tiles per core
CHUNKS = 8       # DMA chunks (NKT/CHUNKS tiles each)
TPC = NKT // CHUNKS
ACT = 100        # action dim
FCN = ACT + 1    # fused FC output cols: [logits | value]
SM_COLS = 224    # smalls tensor cols

F32 = mybir.dt.float32
F32R = mybir.dt.float32r
BF16 = mybir.dt.bfloat16
# Big-matmul operand mode.  "f32r": full f32 storage, float32r matmul
# (1 cycle/row at free-dim>=256, near-f32 accuracy).  "bf16": half the
# HBM traffic but ~1e-2 output error.  The runtime's ~44us collective
# entry barrier hides most of the stream either way, so f32r costs only
# a few us and buys ~4x accuracy margin.
MM_MODE = "f32r"
MM_BF16 = MM_MODE == "bf16"
MMDT = BF16 if MM_BF16 else F32R
# FC path: batch FCB channels per matmul so the free dim clears the
# >=256 threshold where both bf16 and float32r run at 1 cycle/row.
# bf16 FC weights halve the fcw bytes right before the AllReduce doorbell;
# the FC error contribution (~1.8e-3) still leaves 10x gate margin.
FCDT = BF16
FCB = 5                    # channels per FC matmul
FCS = C // FCB             # 12 accumulation steps
# +1 pad column: fp32r matmuls fail the ISA check with an odd free dim.
FCW_FREE = FCB * FCN + 1   # 506 floats = 2024B, fits one PSUM bank
# Wake the collective firmware early with a tiny dummy AllReduce that runs
# under the streaming phase, so the real AllReduce doesn't pay the ~11us
# ncfw wake latency.  (Measured: back-to-back collectives queue badly on
# this stack -- leave off.)
PREWARM_CC = False
# AllReduce dtype for the [100,360] partials: the stage time is ncfw
# per-step dominated, so bf16 doesn't help; keep f32 (exact).
AR_BF16 = False
# K-tiles per DMA chunk: ramp up (TensorEngine starts early) and back down
# (short last-byte -> AllReduce-doorbell path).  Each dma_start costs
# ~0.7-1.1us of sequencer issue time, so chunk issues alternate between the
# two HWDGE engines (sync=SP, scalar=Activation).
CHUNK_SIZES = [2, 4, 8, 10, 12, 12, 8, 4, 4]
assert sum(CHUNK_SIZES) == NKT


def build_nc(debug: bool = False, reps: int = 1):
    nc = bacc.Bacc(
        "TRN2", target_bir_lowering=False, debug=debug, num_devices=N_CORES
    )
    bigbuf = nc.dram_tensor("bigbuf", [128, NKT * BB], MMDT, kind="ExternalInput")
    fcw = nc.dram_tensor("fcw", [N, FCS * FCW_FREE], FCDT, kind="ExternalInput")
    lt = nc.dram_tensor("lt", [N, N], F32, kind="ExternalInput")
    smalls = nc.dram_tensor("smalls", [4, SM_COLS], F32, kind="ExternalInput")
    out_ext = nc.dram_tensor("out", [2, FCN], F32, kind="ExternalOutput")

    with tile.TileContext(nc) as tc:
        with (
            tc.tile_pool(name="big", bufs=1) as bigpool,
            tc.tile_pool(name="wk", bufs=1) as wk,
            tc.tile_pool(name="ps", bufs=1, space="PSUM") as ps,
            tc.tile_pool(name="dram", bufs=1, space="DRAM") as dram,
        ):
            for _rep in range(reps):
                _build_body(nc, bigpool, wk, ps, dram, bigbuf, fcw, lt, smalls, out_ext)

    nc.compile()
    return nc


def _build_body(nc, bigpool, wk, ps, dram, bigbuf, fcw, lt, smalls, out_ext):
    if True:
        if True:
            # Small persistent tensors, issued on the scalar HWDGE queue and
            # order-pinned into the middle of the DMA ramp: early enough to
            # land before the AllReduce window (they'd contend with the
            # collective's SDMA traffic and stall the FC phase), late enough
            # not to delay the first streaming chunk.
            fcw_s = wk.tile([N, FCS * FCW_FREE], FCDT, tag="fcw")
            i_fcw = nc.scalar.dma_start(fcw_s[:], fcw[:, :])
            lt_s = wk.tile([N, N], F32, tag="lt")
            i_lt = nc.scalar.dma_start(lt_s[:], lt[:, :])
            sm_s = wk.tile([4, SM_COLS], F32, tag="smalls")
            i_sm = nc.scalar.dma_start(sm_s[:], smalls[:, :])
            ones_s = wk.tile([1, N], F32, tag="ones")
            nc.any.memset(ones_s[:], 1.0)

            if PREWARM_CC:
                warm_in = dram.tile([1, 8], F32, tag="warmin")
                warm_out = dram.tile([1, 8], F32, tag="warmout")
                warm_sb = wk.tile([1, 8], F32, tag="warmsb")
                nc.any.memset(warm_sb[:], 0.0)
                nc.gpsimd.dma_start(warm_in[:], warm_sb[:])
                nc.gpsimd.collective_compute(
                    "AllReduce",
                    mybir.AluOpType.add,
                    replica_groups=[list(range(N_CORES))],
                    ins=[warm_in.opt()],
                    outs=[warm_out.opt()],
                )

            # Big fused matmul: accumulate all 64 K-tiles into one PSUM bank
            psum_y = ps.tile([N, NW], F32, tag="y")
            mm = 0
            lo = 0
            chunk_dmas = []
            for ch, tpc in enumerate(CHUNK_SIZES):
                bt = bigpool.tile([128, tpc * BB], MMDT, tag=f"chunk{ch}")
                eng = nc.sync if ch % 2 == 0 else nc.scalar
                i_ch = eng.dma_start(bt[:], bigbuf[:, lo : lo + tpc * BB])
                chunk_dmas.append(i_ch)
                lo += tpc * BB
                for t in range(tpc):
                    base = t * BB
                    lhsT = bt[:, base + NW : base + BB]  # [128, 100] xT tile
                    rhs = bt[:, base : base + NW]        # [128, 360] W tile
                    nc.tensor.matmul(
                        psum_y[:], lhsT, rhs,
                        start=(mm == 0), stop=(mm == NKT - 1),
                    )
                    mm += 1
            # Order the scalar-queue issues: small lt/smalls early (the
            # Z-bias matmul wants them), streaming chunks next, and the
            # 2.4MB fcw LAST -- it isn't consumed until the FC phase after
            # the AllReduce, and every byte moved before the doorbell
            # delays the collective on all 8 cores.
            scalar_order = [
                chunk_dmas[1], i_lt, i_sm, chunk_dmas[3],
                chunk_dmas[5], chunk_dmas[7], i_fcw,
            ]
            for a, b in zip(scalar_order[1:], scalar_order):
                add_dep_helper(a.ins, b.ins, False, "scalar DMA queue order")

            # The Chebyshev epilogue is linear, so it commutes with the
            # cross-core sum: compute Z_partial on each core's Y partials
            # BEFORE the collective, then AllReduce only [100,120] (48KB).
            # The bias row is non-zero only in core 0's input data.
            y_part = wk.tile([N, NW], F32, tag="ypart")
            nc.vector.tensor_copy(y_part[:], psum_y[:])
            # U = L@Y2p
            psum_u = ps.tile([N, CB], F32, tag="u")
            nc.tensor.matmul(
                psum_u[:], lt_s[:], y_part[:, 2 * CB : 3 * CB],
                start=True, stop=True,
            )
            # Vin = 2*U + Y1p
            vin_s = wk.tile([N, CB], F32, tag="vin")
            nc.vector.scalar_tensor_tensor(
                vin_s[:], psum_u[:], 2.0, y_part[:, CB : 2 * CB],
                op0=mybir.AluOpType.mult, op1=mybir.AluOpType.add,
            )
            # Z_partial = bias (core 0 only) + L@Vin (+ Y0p - Y2p)
            psum_z = ps.tile([N, CB], F32, tag="z")
            nc.tensor.matmul(
                psum_z[:], ones_s[:], sm_s[0:1, 103 : 103 + CB],
                start=True, stop=False, skip_group_check=True,
            )
            nc.tensor.matmul(
                psum_z[:], lt_s[:], vin_s[:],
                start=False, stop=True, skip_group_check=True,
            )
            d_s = wk.tile([N, CB], F32, tag="d")
            nc.vector.tensor_sub(
                d_s[:], y_part[:, 0:CB], y_part[:, 2 * CB : 3 * CB]
            )
            z_s = wk.tile([N, CB], F32, tag="zs")
            nc.vector.tensor_add(z_s[:], d_s[:], psum_z[:])

            # AllReduce the Z partials across the 8 cores
            ar_in = dram.tile([N, CB], F32, tag="arin")
            ar_out = dram.tile([N, CB], F32, tag="arout")
            nc.sync.dma_start(ar_in[:], z_s[:])
            nc.gpsimd.collective_compute(
                "AllReduce",
                mybir.AluOpType.add,
                replica_groups=[list(range(N_CORES))],
                ins=[ar_in.opt()],
                outs=[ar_out.opt()],
            )
            zall_s = wk.tile([N, CB], F32, tag="zall")
            nc.sync.dma_start(zall_s[:], ar_out[:])

            # emb = tanh(Z)
            emb_s = wk.tile([N, CB], FCDT, tag="emb")
            nc.scalar.activation(
                emb_s[:], zall_s[:], mybir.ActivationFunctionType.Tanh
            )

            # FC heads, FCB channels per accumulating matmul:
            # lhsT = emb[:, (c, 60+c) for c in step] [100, FCB*2]
            # rhs  = fcw[:, step block]              [100, FCB*101]
            # psum block j ([2j:2j+2, j*101:(j+1)*101]) accumulates the
            # (actor, critic) FC partials of channels c = j mod FCB.
            psum_fc = ps.tile([2 * FCB, FCW_FREE], F32, tag="fc")
            for s in range(FCS):
                lhsT = emb_s[:, 2 * FCB * s : 2 * FCB * (s + 1)]
                rhs = fcw_s[:, s * FCW_FREE : (s + 1) * FCW_FREE]
                nc.tensor.matmul(
                    psum_fc[:], lhsT, rhs,
                    start=(s == 0), stop=(s == FCS - 1), skip_group_check=True,
                )
            # extras + bias: lhsT = smalls[:,0:2] [K=4,M=2], rhs = smalls[:,2:103]
            psum_fce = ps.tile([2, FCN], F32, tag="fce")
            nc.tensor.matmul(
                psum_fce[:], sm_s[:, 0:2], sm_s[:, 2 : 2 + FCN],
                start=True, stop=True, skip_group_check=True,
            )
            # Sum the FCB diagonal blocks + extras.  Engine accesses must
            # start at partition 0, so evict PSUM to SBUF, then DMA-fold
            # rows {h, h+2, ...} onto partition h (strided partition reads
            # are fine for DMA), leaving block j at free offset j*505.
            g_s = wk.tile([2 * FCB, FCW_FREE], F32, tag="gs")
            nc.vector.tensor_copy(g_s[:], psum_fc[:])
            g2 = wk.tile([2, FCB * FCW_FREE], F32, tag="g2")
            nc.sync.dma_start(g2[0:1, :], g_s[0 : 2 * FCB : 2, :])
            nc.scalar.dma_start(g2[1:2, :], g_s[1 : 2 * FCB : 2, :])
            fc_s = wk.tile([2, FCN], F32, tag="fcs")
            nc.vector.tensor_add(fc_s[:], g2[:, 0:FCN], psum_fce[:])
            for j in range(1, FCB):
                lo = j * FCW_FREE + j * FCN
                nc.vector.tensor_add(
                    fc_s[:], fc_s[:], g2[:, lo : lo + FCN]
                )
            nc.sync.dma_start(out_ext[:, :], fc_s[:])


def prepare_inputs(
    substrate_features, edge_index, v_cpu_demand_t, v_bw_demand_t,
    num_pending_v_nodes_t, actor_w, actor_b, critic_w, critic_b,
    actor_fc_w, actor_fc_b, critic_fc_w, critic_fc_b,
):
    """Host-side sharding / layout prep. Returns in_maps for the 8 cores."""
    x2 = np.asarray(substrate_features, np.float32)[0]        # [100, F]
    ei = np.asarray(edge_index).astype(np.int64)              # [2, E]
    aw = np.asarray(actor_w, np.float32)                      # [3, F, 60]
    ab = np.asarray(actor_b, np.float32)
    cw = np.asarray(critic_w, np.float32)
    cb = np.asarray(critic_b, np.float32)
    afw = np.asarray(actor_fc_w, np.float32)                  # [6003, 100]
    afb = np.asarray(actor_fc_b, np.float32)
    cfw = np.asarray(critic_fc_w, np.float32)                 # [6003, 1]
    cfb = np.asarray(critic_fc_b, np.float32)
    extras = [
        float(np.asarray(v_cpu_demand_t).reshape(-1)[0]),
        float(np.asarray(v_bw_demand_t).reshape(-1)[0]),
        float(np.asarray(num_pending_v_nodes_t).reshape(-1)[0]),
    ]

    # Dense scaled Laplacian from the edge list (PyG ChebConv, lambda_max=2)
    src, dst = ei[0], ei[1]
    deg = np.bincount(src, minlength=N).astype(np.float32)
    dis = np.where(deg > 0, 1.0 / np.sqrt(np.where(deg > 0, deg, 1.0)), 0.0)
    norm = -(dis[src] * dis[dst]).astype(np.float32)
    L = np.zeros((N, N), np.float32)
    np.add.at(L, (dst, src), norm)
    ltT = np.ascontiguousarray(L.T)                            # lhsT layout

    # Fused conv weights [F, 360]: three Cheb-order blocks of 120 columns;
    # within a block, actor/critic channels pairwise interleaved
    # [a0, c0, a1, c1, ...] so FC lhsT slices of emb are contiguous.
    w_all = np.empty((F, 3, C, 2), np.float32)
    for k in range(3):
        w_all[:, k, :, 0] = aw[k]
        w_all[:, k, :, 1] = cw[k]
    w_all = w_all.reshape(F, NW)
    xT = np.ascontiguousarray(x2.T)                            # [F, 100]

    # FC weights rearranged: fcw[n, c*101 + a] = actor_fc_w[n*60+c, a],
    # col 100 = critic_fc_w[n*60+c, 0]
    A = afw[:6000].reshape(N, C, ACT)
    Cc = cfw[:6000].reshape(N, C, 1)
    fcw_raw = np.concatenate([A, Cc], axis=2).reshape(N, FCS, FCB * FCN)
    fcw_host = np.zeros((N, FCS, FCW_FREE), np.float32)
    fcw_host[:, :, : FCB * FCN] = fcw_raw
    fcw_host = np.ascontiguousarray(fcw_host.reshape(N, FCS * FCW_FREE))
    if FCDT == BF16:
        import ml_dtypes

        fcw_host = fcw_host.astype(ml_dtypes.bfloat16)

    # smalls [4, 224]:
    #  [:, 0:2]      extras lhsT columns (both identical): [v_cpu, v_bw, n_pend, 1]
    #  [:, 2:103]    extras rhs rows: actor_fc_w[6000+j]|critic_fc_w[6000+j];
    #                row 3 = [actor_fc_b | critic_fc_b]
    #  [0, 103:223]  conv bias row [actor_b | critic_b]
    smalls = np.zeros((4, SM_COLS), np.float32)
    for j in range(3):
        smalls[j, 0:2] = extras[j]
        smalls[j, 2 : 2 + ACT] = afw[6000 + j]
        smalls[j, 2 + ACT] = cfw[6000 + j, 0]
    smalls[3, 0:2] = 1.0
    smalls[3, 2 : 2 + ACT] = afb
    smalls[3, 2 + ACT] = cfb[0]
    smalls[0, 103 : 103 + CB] = np.stack([ab, cb], axis=1).reshape(-1)

    in_maps = []
    for m in range(N_CORES):
        sl = slice(m * FS, (m + 1) * FS)
        big = np.concatenate([w_all[sl], xT[sl]], axis=1)      # [8192, 460]
        big = np.ascontiguousarray(
            big.reshape(NKT, KT, BB).transpose(1, 0, 2).reshape(128, NKT * BB)
        )
        if MM_BF16:
            import ml_dtypes

            big = big.astype(ml_dtypes.bfloat16)
        # The conv bias flows through the Z AllReduce -- only core 0 adds it.
        sm_m = smalls.copy()
        if m > 0:
            sm_m[0, 103 : 103 + CB] = 0.0
        in_maps.append(
            {"bigbuf": big, "fcw": fcw_host, "lt": ltT, "smalls": sm_m}
        )
    return in_maps


def unshard(results):
    out = np.asarray(results[0]["out"], np.float32)            # [2, 101]
    logits = np.ascontiguousarray(out[0:1, 0:ACT])             # [1, 100]
    values = np.ascontiguousarray(out[1:2, ACT : ACT + 1])     # [1, 1]
    return logits, values


_CACHED = {}


def kernel(**inputs):
    from concourse.bass_utils import run_bass_kernel_spmd

    in_maps = prepare_inputs(**inputs)
    if "nc" not in _CACHED:
        _CACHED["nc"] = build_nc(debug=False)
    res = run_bass_kernel_spmd(
        _CACHED["nc"], in_maps, core_ids=list(range(N_CORES))
    )
    return unshard(res.results)


def run_profiled(in_maps, tmpdir=None, trace=False):
    """Like kernel(), but optionally with NTFF profiling."""
    from concourse.bass_utils import run_bass_kernel_spmd

    if "nc" not in _CACHED:
        _CACHED["nc"] = build_nc(debug=False)
    res = run_bass_kernel_spmd(
        _CACHED["nc"], in_maps, core_ids=list(range(N_CORES)),
        trace=trace, tmpdir=tmpdir,
    )
    return unshard(res.results), res.exec_time_ns, res
